# revision 39
# baseline (speedup 1.0000x reference)
"""Trainium2 Bass kernel for nn_AttentionNetwork (ragged path attention).

Data-parallel over 8 NeuronCores: 512 paths per core, dealt round-robin by
global length-sorted rank so all 8 cores see near-identical sorted length
profiles. Paths are packed into variable-width blocks (bp paths x cap
node-slots, cap % 4 == 0, bp*cap <= 512), emitted smallest-cap-first.

v2 fast path (zero biases, which setup_inputs produces):
- Stage 1 per block: bf16 MLP matmuls into paired-PSUM h tiles
  [128, 2, rows] (one merged relu ACT instr per pair), scores as four
  [128,1]-stationary bf16 matmuls into PSUM, exp -> erow (bf16, kept in a
  [1, TOT] strip and DMA'd out), gpsimd partition_broadcast of the weight
  row, 4x DVE x*w muls, then a bf16 halving-add tree all the way to 1
  element whose final f32 add writes straight into the persistent pfall
  tile (no TENSOR_REDUCE, no normalize).
- Normalization is done on the HOST: relu(c*z) = c*relu(z) for c>0 and
  ab1 == 0, so stage 2 consumes unnormalized path features; the host
  divides the resulting path logits (and features) by the per-path
  denominator, which it computes itself in f64 from the shipped
  exp-score rows masked by the true lengths (this also removes all
  npad/padded-slot correction machinery: padded x rows are zero, so they
  contribute exactly 0 to the numerator).
- Stage 2 (f32r) is chunked into 4x128 paths and emitted mid-stage-1 in
  block-emit order so it overlaps; pf chunks DMA out as they finish.

Legacy path (nonzero pb1/ab1): the previous fully-general kernel.

Measured on 8x trn2: baseline 197 us -> v2 ~? us, rel err ~2-3e-3.
"""

import sys

if "/opt/trn_rl_repo" not in sys.path:
    sys.path.insert(0, "/opt/trn_rl_repo")

from contextlib import ExitStack

import ml_dtypes
import numpy as np

import concourse.bass as bass  # noqa: F401
import concourse.mybir as mybir
import concourse.tile as tile
from concourse import bacc, bass_utils

P, LMAX, D, H = 4096, 64, 512, 512
NCORES = 8
PS = P // NCORES          # paths per core
KC = D // 128             # contraction chunks
HC = H // 128             # hidden chunks
ROWS_TARGET = 512         # max rows (bp*cap) per block

f32 = mybir.dt.float32
f32r = mybir.dt.float32r
bf16 = mybir.dt.bfloat16
AF = mybir.ActivationFunctionType
ALU = mybir.AluOpType
AX = mybir.AxisListType

LAST_RESULT = None
_PROG_CACHE = {}
_TRACE_KW = {}


def _make_blocks(len_max, cap_mult, rows_mult=1):
    """Greedy pack sorted-desc lengths into (bp, cap) blocks, bp*cap<=512,
    bp*cap % rows_mult == 0."""
    blocks = []
    i = 0
    while i < PS:
        cap = int(len_max[i])
        cap += (-cap) % cap_mult       # round cap up to a multiple
        from math import gcd
        need = rows_mult // gcd(cap, rows_mult)   # bp must be a multiple
        bp = min(ROWS_TARGET // cap // need * need, PS - i)
        if bp <= 0 or (bp * cap) % rows_mult != 0:
            # remainder tail: bump cap until bp*cap is aligned
            bp = PS - i
            while (bp * cap) % rows_mult != 0:
                cap += cap_mult
        blocks.append((bp, cap))
        i += bp
    return tuple(blocks)


# --------------------------------------------------------------------------
# v2 fast path (pb1 == 0 and ab1 == 0)
# --------------------------------------------------------------------------

S2_CSZ = 128              # stage-2 chunk width (paths)


def _build_program_v2(blocks):
    """blocks: tuple of (bp, cap) in EMIT order; cap % 4 == 0."""
    nb = len(blocks)
    rows_list = [bp * cap for bp, cap in blocks]
    tot_rows = sum(rows_list)
    NCH = KC + 1              # 4 x*w channels + 1 weight channel

    nc = bacc.Bacc("TRN2", target_bir_lowering=False, debug=False, num_devices=NCORES)

    fp8 = mybir.dt.float8e4
    DR = mybir.MatmulPerfMode.DoubleRow
    xb = nc.dram_tensor("xb", [KC * 128 * tot_rows], bf16, kind="ExternalInput")
    xb8 = nc.dram_tensor("xb8", [KC * 128 * tot_rows], fp8, kind="ExternalInput")
    w1 = nc.dram_tensor("w1", [128, KC * H], fp8, kind="ExternalInput")
    w2 = nc.dram_tensor("w2", [128, HC], bf16, kind="ExternalInput")
    aw1 = nc.dram_tensor("aw1", [128, KC * H], f32r, kind="ExternalInput")
    aw2 = nc.dram_tensor("aw2", [128, HC], f32r, kind="ExternalInput")
    out_pf = nc.dram_tensor("out_pf", [128, KC * PS], f32, kind="ExternalOutput")
    out_au = nc.dram_tensor("out_au", [1, PS], f32, kind="ExternalOutput")
    out_er = nc.dram_tensor("out_er", [1, tot_rows], bf16, kind="ExternalOutput")

    x_offs = [0] * nb
    p_offs = [0] * nb
    r_offs = [0] * nb
    acc_x = acc_p = acc_r = 0
    for i in range(nb):
        x_offs[i], p_offs[i], r_offs[i] = acc_x, acc_p, acc_r
        acc_x += KC * 128 * rows_list[i]
        acc_p += blocks[i][0]
        acc_r += rows_list[i]
    assert acc_p == PS

    with ExitStack() as ctx:
        tc = ctx.enter_context(tile.TileContext(nc))
        const = ctx.enter_context(tc.tile_pool(name="const", bufs=1))
        xpool = ctx.enter_context(tc.tile_pool(name="x", bufs=4))
        xwpool = ctx.enter_context(tc.tile_pool(name="xw", bufs=3))
        hpool = ctx.enter_context(tc.tile_pool(name="h", bufs=3))
        vpool = ctx.enter_context(tc.tile_pool(name="v", bufs=2))
        ph_pool = ctx.enter_context(tc.tile_pool(name="ph", bufs=2, space="PSUM"))
        ps_pool = ctx.enter_context(tc.tile_pool(name="ps", bufs=2, space="PSUM"))
        s2_pool = ctx.enter_context(tc.tile_pool(name="s2", bufs=2, space="PSUM"))

        # ACT table prefetch: no DMA dependency, fires at t=0
        t_one = const.tile([1, 1], f32)
        nc.vector.memset(t_one[:], 1.0)
        t_warm = const.tile([1, 1], f32)
        nc.scalar.activation(t_warm[:], t_one[:], AF.Exp)

        t_w1 = const.tile([128, KC, H], fp8)
        nc.sync.dma_start(t_w1[:].rearrange("d k h -> d (k h)"), w1.ap())
        t_w2 = const.tile([128, HC], bf16)
        nc.sync.dma_start(t_w2[:], w2.ap())
        t_aw1 = const.tile([128, KC, H], f32r)
        t_aw2 = const.tile([128, HC], f32r)

        pfall = const.tile([128, KC, PS], f32r)  # unnormalized path features
        pfall_r = pfall[:]
        er_all = const.tile([1, tot_rows], bf16)  # exp(score) rows
        au_all = const.tile([1, PS], f32)         # unnormalized path logits
        max_nseg = max(KC * bp for bp, _ in blocks)
        t_zero = const.tile([128, max_nseg], bf16)
        nc.vector.memset(t_zero[:], 0.0)

        def emit_scores(st):
            rows, r_off = st["rows"], st["r_off"]
            b, rh = st["b"], st["rh"]
            ps_s = ps_pool.tile([1, rows], f32, tag="s", name=f"ps_{b}")
            for j in range(HC):
                nc.tensor.matmul(
                    ps_s[:], t_w2[:, j : j + 1], rh[:, j, :],
                    start=(j == 0), stop=(j == HC - 1),
                )
            nc.scalar.activation(er_all[0:1, r_off : r_off + rows], ps_s[:], AF.Exp)

        def emit_gtail(g):
            """bcast/mul/tree for a group of same-cap blocks sharing one
            x_b tile; one DVE instruction per op over the whole group."""
            cap, bp, rows = g["cap"], g["bp"], g["rows"]
            p_off, r_off, x_g = g["p_off"], g["r_off"], g["x_g"]
            gi = g["gi"]
            # xwt channels 0..KC-1 = x*w ; channel KC = w = exp(scores)
            xwt = xwpool.tile([128, NCH, rows], bf16, tag="xw", name=f"xw_{gi}")
            nc.gpsimd.partition_broadcast(
                xwt[:, KC, :], er_all[0:1, r_off : r_off + rows]
            )
            w_bc = xwt[:, KC, :].rearrange("p (u r) -> p u r", u=1).to_broadcast(
                [128, KC, rows]
            )
            nc.vector.tensor_mul(xwt[:, 0:KC, :], x_g[:], w_bc)

            # segmented sum over cap: bf16 halving adds (2x DVE) down to 2-3,
            # then final f32 add(s) straight into pfall; odd level widths
            # fold the leftover column with a zero-add
            nseg = KC * bp
            zcol = t_zero[:, 0:nseg].rearrange("p (f x) -> p f x", x=1)
            cur_ap = xwt[:, 0:KC, :].rearrange("p f (s l) -> p (f s) l", l=cap)
            # small tree levels go to the (mostly idle) gpsimd engine —
            # except near the kernel end, where DVE latency is shorter
            offload = False  # gpsimd tensor ops measured far too slow
            cc = cap
            lvl = 0
            while cc > 3:
                half = cc // 2
                odd = cc % 2
                eng = nc.gpsimd if (offload and nseg * (half + odd) <= 448) \
                    else nc.vector
                nxt = vpool.tile(
                    [128, nseg * (half + odd)], bf16,
                    tag=f"hv{lvl}", name=f"hv{lvl}_{gi}",
                )
                nxt_ap = nxt[:].rearrange("p (f l) -> p f l", l=half + odd)
                eng.tensor_add(
                    nxt_ap[:, :, 0:half], cur_ap[:, :, 0:half],
                    cur_ap[:, :, half : 2 * half],
                )
                if odd:
                    eng.tensor_add(
                        nxt_ap[:, :, half : half + 1],
                        cur_ap[:, :, 2 * half : 2 * half + 1], zcol,
                    )
                cur_ap = nxt_ap
                cc = half + odd
                lvl += 1
            eng = nc.gpsimd if offload else nc.vector
            out_ap = pfall[:, :, p_off : p_off + bp]
            in0 = cur_ap[:, :, 0].rearrange("p (f s) -> p f s", f=KC)
            in1 = cur_ap[:, :, 1].rearrange("p (f s) -> p f s", f=KC)
            if cc == 2:
                eng.tensor_add(out_ap, in0, in1)
            else:
                tmp = vpool.tile([128, nseg], bf16, tag="hvf", name=f"hvf_{gi}")
                tmp_ap = tmp[:].rearrange("p (f s) -> p f s", f=KC)
                eng.tensor_add(tmp_ap, in0, in1)
                in2 = cur_ap[:, :, 2].rearrange("p (f s) -> p f s", f=KC)
                eng.tensor_add(out_ap, tmp_ap, in2)

        def emit_s2_chunk(ci, c0, csz):
            ph2 = s2_pool.tile(
                [128, HC, csz], f32, tag="s2c", name=f"ph2_{ci}",
                padded_shape=[128, HC, S2_CSZ],
            )
            for j in range(HC):
                for k in range(KC):
                    nc.tensor.matmul(
                        ph2[:, j, :],
                        t_aw1[:, k, 128 * j : 128 * (j + 1)],
                        pfall_r[:, k, c0 : c0 + csz],
                        start=(k == 0),
                        stop=(k == KC - 1),
                        skip_group_check=True,
                    )
            rh2 = hpool.tile([128, HC, csz], f32r, tag="rh2", name=f"rh2_{ci}")
            nc.scalar.activation(rh2[:], ph2[:], AF.Relu)
            psa = s2_pool.tile(
                [128, HC, csz], f32, tag="s2c", name=f"psa_{ci}",
                padded_shape=[128, HC, S2_CSZ],
            )
            for j in range(HC):
                nc.tensor.matmul(
                    psa[0:1, 0, :], t_aw2[:, j : j + 1], rh2[:, j, :],
                    start=(j == 0), stop=(j == HC - 1),
                    skip_group_check=True,
                )
            nc.scalar.copy(au_all[0:1, c0 : c0 + csz], psa[0:1, 0, :])
            nc.sync.dma_start(
                out_pf.ap().rearrange("d (k p) -> d k p", k=KC)[:, :, c0 : c0 + csz],
                pfall[:].bitcast(f32)[:, :, c0 : c0 + csz],
            )

        # stage-2 chunks: last one small so the final serial tail is short
        chunk_list = [(0, 128), (128, 128), (256, 128), (384, 96), (480, 32)]

        # group consecutive same-cap blocks for a shared tail; keep the
        # last blocks single so the end-of-kernel tail chains stay short
        groups = []
        cur = [0]
        for b in range(1, nb):
            if (blocks[b][1] == blocks[cur[-1]][1] and len(cur) < 4
                    and b < nb - 4
                    and sum(rows_list[i] for i in cur) + rows_list[b] <= 1024):
                cur.append(b)
            else:
                groups.append(cur)
                cur = [b]
        groups.append(cur)
        grp_of = {}
        for gi, g in enumerate(groups):
            for b in g:
                grp_of[b] = gi

        gstate = [None] * len(groups)
        pend = None          # block whose scores/x-DMA are one block delayed
        next_chunk = 0
        tail_ends = [0, 0, 0]  # emitted-tail path marks; chunks trail 2 groups
        deferred_xb = []

        def emit_xb_dma(bb, goff, rows):
            gg = gstate[grp_of[bb]]
            nc.sync.dma_start(
                gg["x_g"][:, :, goff : goff + rows],
                xb.ap()[
                    x_offs[bb] : x_offs[bb] + KC * 128 * rows
                ].rearrange("(d k r) -> d k r", k=KC, d=128),
            )

        def flush_pend(pend):
            bb = pend["b"]
            gg = gstate[grp_of[bb]]
            if bb < 4:
                # keep the startup DMA queue clear for fp8 x blocks; the
                # group tail needs the DMA, so it defers too
                deferred_xb.append(("dma", (bb, pend["goff"], pend["rows"])))
            else:
                emit_xb_dma(bb, pend["goff"], pend["rows"])
            emit_scores(pend)
            if bb == gg["last"]:
                if bb < 4:
                    deferred_xb.append(("tail", gg))
                else:
                    emit_gtail(gg)
                tail_ends.append(gg["p_off"] + gg["bp"])

        for b in range(nb):
            bp, cap = blocks[b]
            rows = rows_list[b]

            assert rows % 16 == 0
            x_8 = xpool.tile([128, KC, rows], fp8, tag="x8", name=f"x8_{b}")
            nc.sync.dma_start(
                x_8[:],
                xb8.ap()[x_offs[b] : x_offs[b] + KC * 128 * rows].rearrange(
                    "(d k r) -> d k r", k=KC, d=128
                ),
            )

            rh = hpool.tile([128, HC, rows], bf16, tag="rh", name=f"rh_{b}")
            for t in range(HC // 2):
                ph = ph_pool.tile(
                    [128, 2, rows], f32, tag="h", name=f"ph{t}_{b}",
                    padded_shape=[128, 2, 512],
                )
                for jj in range(2):
                    j = 2 * t + jj
                    for u in range(KC // 2):
                        nc.tensor.matmul(
                            ph[:, jj, :],
                            t_w1[:, 2 * u : 2 * u + 2, 128 * j : 128 * (j + 1)],
                            x_8[:, 2 * u : 2 * u + 2, :],
                            start=(u == 0),
                            stop=(u == KC // 2 - 1),
                            perf_mode=DR,
                            skip_group_check=True,
                        )
                # undo the x32 weight scale exactly (power of 2)
                nc.scalar.activation(
                    rh[:, 2 * t : 2 * t + 2, :], ph[:], AF.Relu, scale=1.0 / 32.0
                )

            gi = grp_of[b]
            if gstate[gi] is None:
                gbs = groups[gi]
                g_rows = sum(rows_list[i] for i in gbs)
                x_g = xpool.tile(
                    [128, KC, g_rows], bf16, tag="xb", name=f"xg_{gi}"
                )
                gstate[gi] = {
                    "gi": gi, "cap": cap,
                    "bp": sum(blocks[i][0] for i in gbs),
                    "rows": g_rows, "p_off": p_offs[gbs[0]],
                    "r_off": r_offs[gbs[0]], "x_g": x_g,
                    "last": gbs[-1], "goff": 0,
                    "offload": gi < len(groups) - 5,
                }
            g = gstate[gi]

            if b == 5:
                for kind, payload in deferred_xb:
                    if kind == "dma":
                        emit_xb_dma(*payload)
                    else:
                        emit_gtail(payload)
                deferred_xb.clear()

            if pend is not None:
                flush_pend(pend)
            pend = {"b": b, "rows": rows, "r_off": r_offs[b], "rh": rh,
                    "goff": g["goff"]}
            g["goff"] += rows
            if 8 <= b < 8 + KC:
                k = b - 8
                nc.sync.dma_start(
                    t_aw1[:, k, :],
                    aw1.ap().rearrange("d (k h) -> d k h", k=KC)[:, k, :],
                )
                if k == KC - 1:
                    nc.sync.dma_start(t_aw2[:], aw2.ap())

            if b >= 12:
                while next_chunk < len(chunk_list):
                    c0, csz = chunk_list[next_chunk]
                    if c0 + csz > tail_ends[-3]:
                        break
                    emit_s2_chunk(next_chunk, c0, csz)
                    next_chunk += 1

        flush_pend(pend)
        nc.sync.dma_start(out_er.ap(), er_all[:])
        while next_chunk < len(chunk_list):
            c0, csz = chunk_list[next_chunk]
            emit_s2_chunk(next_chunk, c0, csz)
            next_chunk += 1

        nc.sync.dma_start(out_au.ap(), au_all[:])

    nc.compile()
    return nc


def _prep_v2(inputs):
    x = np.asarray(inputs["paths_nodes"], dtype=np.float32)
    lengths = np.asarray(inputs["lengths"], dtype=np.int32)
    pW1 = np.asarray(inputs["pW1"], dtype=np.float32)
    pw2 = np.asarray(inputs["pw2"], dtype=np.float32)
    aW1 = np.asarray(inputs["aW1"], dtype=np.float32)
    aw2 = np.asarray(inputs["aw2"], dtype=np.float32)
    # pb2 / ab2 shift their softmax logits uniformly -> no effect on output.

    bf = ml_dtypes.bfloat16
    # Deal paths round-robin by global sorted rank: core c gets ranks c, c+8, ...
    order_g = np.argsort(-lengths, kind="stable")          # [P] desc
    orders = order_g.reshape(PS, NCORES).T                 # [NC, PS]
    sorted_len = lengths[orders]                           # [NC, PS] desc per core
    len_max = sorted_len.max(axis=0)                       # [PS]
    blocks_sorted = _make_blocks(len_max, 2, rows_mult=16)  # fp8 DR needs %16
    # emit order: smallest-cap block first, then the rest largest-first
    nbs = len(blocks_sorted)
    emit_idx = [nbs - 1] + list(range(nbs - 1))
    seg_starts0 = np.concatenate([[0], np.cumsum([b[0] for b in blocks_sorted])])
    blocks = []
    seg_starts = []
    for n, i in enumerate(emit_idx):
        bp, cap = blocks_sorted[i]
        s0 = seg_starts0[i]
        if n == len(emit_idx) - 1 and bp >= 32:
            # quarter-split the final emitted block: its tail chain is the
            # serial end-of-kernel critical path
            h = bp // 4 // 8 * 8
            blocks += [(bp - 3 * h, cap), (h, cap), (h, cap), (h, cap)]
            seg_starts += [s0, s0 + bp - 3 * h, s0 + bp - 2 * h, s0 + bp - h]
        elif n >= len(emit_idx) - 3 and bp >= 16:
            h = bp // 2 // 8 * 8
            blocks += [(bp - h, cap), (h, cap)]
            seg_starts += [s0, s0 + (bp - h)]
        else:
            blocks.append((bp, cap))
            seg_starts.append(s0)
    blocks = tuple(blocks)

    e4 = ml_dtypes.float8_e4m3
    w1_128 = np.ascontiguousarray(
        pW1.reshape(KC, 128, H).transpose(1, 0, 2).reshape(128, KC * H)
    ) * 32.0
    w2_np = np.ascontiguousarray(pw2.reshape(HC, 128).T).astype(bf)
    aw1_np = np.ascontiguousarray(
        aW1.reshape(KC, 128, H).transpose(1, 0, 2).reshape(128, KC * H)
    ).astype(np.float32)
    aw2_np = np.ascontiguousarray(aw2.reshape(HC, 128).T).astype(np.float32)

    ar = np.arange(LMAX + 4)
    in_maps = []
    blk_lens = []   # per core: list of per-block length arrays (device order)
    for c in range(NCORES):
        # per-core stochastic rounding of W1 to fp8: quantization error is
        # zero-mean and independent across cores, so its (otherwise
        # systematic) effect on the final path-average drops by ~sqrt(8)
        rng = np.random.default_rng(c)
        eps = np.abs(np.spacing(np.abs(w1_128).astype(e4))).astype(np.float32)
        u = rng.random(w1_128.shape, dtype=np.float32) - 0.5
        w1_np = (w1_128 + u * eps).astype(e4)

        xc = x[orders[c]]                             # [PS, LMAX, D] sorted
        lc = sorted_len[c]                            # [PS]
        xr_parts = []
        x8_parts = []
        lens_c = []
        for s0, (bp, cap) in zip(seg_starts, blocks):
            lb = lc[s0 : s0 + bp]
            lens_c.append(lb.copy())
            ccap = min(cap, LMAX)
            xblk = xc[s0 : s0 + bp, :ccap, :]         # [bp, ccap, D]
            mask = ar[None, :ccap, None] < lb[:, None, None]
            xblk = np.where(mask, xblk, 0.0).astype(np.float32)
            if ccap < cap:                            # pad slot(s)
                pad = np.zeros((bp, cap - ccap, D), dtype=np.float32)
                xblk = np.concatenate([xblk, pad], axis=1)
            xb_t = (
                xblk.reshape(bp, cap, KC, 128)
                .transpose(3, 2, 0, 1)                # (d, k, path, slot)
                .reshape(128, KC * bp * cap)
            )
            xr_parts.append(xb_t.astype(bf).ravel())
            x8_parts.append(xb_t.astype(e4).ravel())
        in_maps.append(
            {
                "xb": np.concatenate(xr_parts),
                "xb8": np.concatenate(x8_parts),
                "w1": w1_np,
                "w2": w2_np,
                "aw1": aw1_np,
                "aw2": aw2_np,
            }
        )
        blk_lens.append(lens_c)
    return blocks, in_maps, blk_lens


def _kernel_v2(inputs):
    global LAST_RESULT
    blocks, in_maps, blk_lens = _prep_v2(inputs)
    key = ("v2", blocks)
    if key not in _PROG_CACHE:
        _PROG_CACHE[key] = _build_program_v2(blocks)
    nc = _PROG_CACHE[key]

    res = bass_utils.run_bass_kernel_spmd(
        nc, in_maps, core_ids=list(range(NCORES)), **_TRACE_KW
    )
    LAST_RESULT = res

    tot_rows = sum(bp * cap for bp, cap in blocks)
    a_all = []
    winv_all = []
    pf_all = []
    for c, r in enumerate(res.results):
        pf = r["out_pf"].reshape(128, KC, PS)
        au = r["out_au"].reshape(PS).astype(np.float64)
        er = r["out_er"].reshape(tot_rows).astype(np.float64)
        wsum = np.empty(PS, dtype=np.float64)
        p = rr = 0
        for (bp, cap), lb in zip(blocks, blk_lens[c]):
            eb = er[rr : rr + bp * cap].reshape(bp, cap)
            m = np.arange(cap)[None, :] < lb[:, None]
            wsum[p : p + bp] = np.where(m, eb, 0.0).sum(axis=1)
            p += bp
            rr += bp * cap
        a_all.append(au / wsum)
        winv_all.append(1.0 / wsum)
        pf_all.append(pf)
    a_cat = np.concatenate(a_all)
    m = a_cat.max()
    vec = np.zeros((128, KC), dtype=np.float64)
    denom = 0.0
    for c in range(NCORES):
        aw = np.exp(a_all[c] - m)
        denom += aw.sum()
        scale = aw * winv_all[c]
        vec += (pf_all[c].astype(np.float64) * scale[None, None, :]).sum(axis=2)
    user = np.ascontiguousarray(vec.T).reshape(D) / denom
    return user.astype(np.float32)


# --------------------------------------------------------------------------
# legacy fully-general path (nonzero pb1/ab1) — previous kernel, unchanged
# --------------------------------------------------------------------------

def _build_program_legacy(blocks):
    """blocks: tuple of (bp, cap); one block = bp paths x cap node slots."""
    nb = len(blocks)
    rows_list = [bp * cap for bp, cap in blocks]
    tot_rows = sum(rows_list)
    NCH = KC + 1              # 4 x*w channels + 1 weight channel

    nc = bacc.Bacc("TRN2", target_bir_lowering=False, debug=False, num_devices=NCORES)

    xb = nc.dram_tensor("xb", [KC * 128 * tot_rows], bf16, kind="ExternalInput")
    npad = nc.dram_tensor("npad", [128, PS], f32, kind="ExternalInput")
    w1 = nc.dram_tensor("w1", [128, KC * H], bf16, kind="ExternalInput")
    w2 = nc.dram_tensor("w2", [128, HC], bf16, kind="ExternalInput")
    b1 = nc.dram_tensor("b1", [128, HC], f32, kind="ExternalInput")
    aw1 = nc.dram_tensor("aw1", [128, KC * H], f32r, kind="ExternalInput")
    ab1 = nc.dram_tensor("ab1", [128, HC], f32, kind="ExternalInput")
    aw2 = nc.dram_tensor("aw2", [128, HC], f32r, kind="ExternalInput")
    one1_bf = nc.dram_tensor("one1_bf", [1, 1], bf16, kind="ExternalInput")
    out_pf = nc.dram_tensor("out_pf", [128, KC * PS], f32, kind="ExternalOutput")
    out_ea = nc.dram_tensor("out_ea", [1, PS], f32, kind="ExternalOutput")
    out_stats = nc.dram_tensor("out_stats", [1, 2], f32, kind="ExternalOutput")

    with ExitStack() as ctx:
        tc = ctx.enter_context(tile.TileContext(nc))
        const = ctx.enter_context(tc.tile_pool(name="const", bufs=1))
        xpool = ctx.enter_context(tc.tile_pool(name="x", bufs=4))
        xwpool = ctx.enter_context(tc.tile_pool(name="xw", bufs=3))
        hpool = ctx.enter_context(tc.tile_pool(name="h", bufs=3))
        vpool = ctx.enter_context(tc.tile_pool(name="v", bufs=2))
        spool = ctx.enter_context(tc.tile_pool(name="s", bufs=3))
        ph_pool = ctx.enter_context(tc.tile_pool(name="ph", bufs=6, space="PSUM"))
        ps_pool = ctx.enter_context(tc.tile_pool(name="ps", bufs=2, space="PSUM"))

        t_w1 = const.tile([128, KC, H], bf16)
        for k in range(KC):
            nc.sync.dma_start(
                t_w1[:, k, :], w1.ap().rearrange("d (k h) -> d k h", k=KC)[:, k, :]
            )
        t_w2 = const.tile([128, HC], bf16)
        nc.sync.dma_start(t_w2[:], w2.ap())
        t_b1 = const.tile([128, HC], f32)
        nc.sync.dma_start(t_b1[:], b1.ap())
        t_npad = const.tile([128, PS], f32)
        t_one1 = const.tile([1, 1], bf16)
        nc.sync.dma_start(t_one1[:], one1_bf.ap())
        t_warm = const.tile([1, 1], f32)
        nc.scalar.activation(t_warm[:], t_one1[:], AF.Exp)
        t_aw1 = const.tile([128, KC, H], f32r)
        t_ab1 = const.tile([128, HC], f32)
        t_aw2 = const.tile([128, HC], f32r)

        pfT = const.tile([128, KC, PS], f32r)  # normalized path features

        x_offs = [0] * nb
        p_offs = [0] * nb
        acc_x = acc_p = 0
        for i in range(nb):
            x_offs[i], p_offs[i] = acc_x, acc_p
            acc_x += KC * 128 * rows_list[i]
            acc_p += blocks[i][0]
        assert acc_p == PS

        def emit_tail(st):
            bp, cap, rows, p_off = st["bp"], st["cap"], st["rows"], st["p_off"]
            b, x_b, rh = st["b"], st["x_b"], st["rh"]
            ps_s = ps_pool.tile([1, rows], f32, tag="s", name=f"ps_{b}")
            for j in range(HC):
                nc.tensor.matmul(
                    ps_s[:], t_w2[:, j : j + 1], rh[:, j, :],
                    start=(j == 0), stop=(j == HC - 1),
                )
            erow = spool.tile([1, rows], bf16, tag="erow", name=f"er_{b}")
            nc.scalar.activation(erow[:], ps_s[:], AF.Exp)

            xwt = xwpool.tile([128, NCH, rows], bf16, tag="xw", name=f"xw_{b}")
            nc.gpsimd.partition_broadcast(xwt[:, KC, :], erow[:])
            for k in range(KC):
                nc.vector.tensor_mul(xwt[:, k, :], x_b[:, k, :], xwt[:, KC, :])

            nseg = NCH * bp
            cur_ap = xwt[:].rearrange("p f (s l) -> p (f s) l", l=cap)
            cc = cap
            lvl = 0
            while cc % 2 == 0 and cc > 2:
                half = cc // 2
                nxt = vpool.tile(
                    [128, nseg * half], bf16, tag=f"hv{lvl}", name=f"hv{lvl}_{b}"
                )
                nxt_ap = nxt[:].rearrange("p (f l) -> p f l", l=half)
                nc.vector.tensor_add(
                    nxt_ap, cur_ap[:, :, 0:half], cur_ap[:, :, half:cc]
                )
                cur_ap = nxt_ap
                cc = half
                lvl += 1
            praw = spool.tile([128, NCH * bp], f32, tag="praw", name=f"praw_{b}")
            praw_seg = praw[:].rearrange("p (f s) -> p f s", f=NCH)
            nc.vector.reduce_sum(praw[:], cur_ap, axis=AX.X)

            wcor = spool.tile([128, bp], f32, tag="wcor", name=f"wcor_{b}")
            nc.vector.tensor_sub(
                wcor[:], praw_seg[:, KC, :], t_npad[:, p_off : p_off + bp]
            )
            winv = spool.tile([128, bp], f32, tag="winv", name=f"winv_{b}")
            nc.vector.reciprocal(winv[:], wcor[:])
            winv_bc = winv[:].rearrange("p (x s) -> p x s", x=1).to_broadcast(
                [128, KC, bp]
            )
            nc.vector.tensor_mul(
                pfT[:, :, p_off : p_off + bp], praw_seg[:, 0:KC, :], winv_bc
            )

        emit_order = [nb - 1] + list(range(nb - 1))
        prev = None
        for ei, b in enumerate(emit_order):
            bp, cap = blocks[b]
            rows = rows_list[b]
            x_off, p_off = x_offs[b], p_offs[b]

            x_b = xpool.tile([128, KC, rows], bf16, tag="xb", name=f"xb_{b}")
            nc.sync.dma_start(
                x_b[:],
                xb.ap()[x_off : x_off + KC * 128 * rows].rearrange(
                    "(k d r) -> d k r", k=KC, d=128
                ),
            )

            rh = hpool.tile([128, HC, rows], bf16, tag="rh", name=f"rh_{b}")
            for j in range(HC):
                ph = ph_pool.tile([128, rows], f32, tag="h", name=f"ph{j}_{b}")
                for k in range(KC):
                    nc.tensor.matmul(
                        ph[:],
                        t_w1[:, k, 128 * j : 128 * (j + 1)],
                        x_b[:, k, :],
                        start=(k == 0),
                        stop=(k == KC - 1),
                    )
                nc.scalar.activation(
                    rh[:, j, :], ph[:], AF.Relu, bias=t_b1[:, j : j + 1]
                )

            if prev is not None:
                emit_tail(prev)
            prev = {"b": b, "bp": bp, "cap": cap, "rows": rows,
                    "p_off": p_off, "x_b": x_b, "rh": rh}

            if ei == 0:
                nc.sync.dma_start(t_npad[:], npad.ap())
            if ei == 1:
                nc.sync.dma_start(t_aw1[:].rearrange("d k h -> d (k h)"), aw1.ap())
                nc.sync.dma_start(t_ab1[:], ab1.ap())
                nc.sync.dma_start(t_aw2[:], aw2.ap())
        emit_tail(prev)
        nc.sync.dma_start(
            out_pf.ap(), pfT[:].bitcast(f32).rearrange("d k p -> d (k p)")
        )

        pfr = pfT[:]
        halves = [(0, PS // 2), (PS // 2, PS // 2)]
        rh2_list = []
        for j in range(HC):
            ph2 = ph_pool.tile([128, PS], f32, tag="h")
            for (h0, hsz) in halves:
                for k in range(KC):
                    nc.tensor.matmul(
                        ph2[:, h0 : h0 + hsz],
                        t_aw1[:, k, 128 * j : 128 * (j + 1)],
                        pfr[:, k, h0 : h0 + hsz],
                        start=(k == 0),
                        stop=(k == KC - 1),
                        skip_group_check=True,
                    )
            rh2 = hpool.tile([128, PS], f32r, tag=f"rh2{j}")
            nc.scalar.activation(rh2[:], ph2[:], AF.Relu, bias=t_ab1[:, j : j + 1])
            rh2_list.append(rh2)

        ps_a = ps_pool.tile([1, PS], f32, tag="s")
        for j in range(HC):
            nc.tensor.matmul(
                ps_a[:], t_aw2[:, j : j + 1], rh2_list[j][:],
                start=(j == 0), stop=(j == HC - 1),
            )

        ea = spool.tile([1, PS], f32, tag="ea")
        s_t = spool.tile([1, 1], f32, tag="s1")
        nc.scalar.activation(ea[:], ps_a[:], AF.Exp, accum_out=s_t[:])
        nc.sync.dma_start(out_ea.ap(), ea[:])
        nc.sync.dma_start(out_stats.ap()[:, 0:1], s_t[:])
        nc.sync.dma_start(out_stats.ap()[:, 1:2], s_t[:])

    nc.compile()
    return nc


def _prep_legacy(inputs):
    x = np.asarray(inputs["paths_nodes"], dtype=np.float32)
    lengths = np.asarray(inputs["lengths"], dtype=np.int32)
    pW1 = np.asarray(inputs["pW1"], dtype=np.float32)
    pb1 = np.asarray(inputs["pb1"], dtype=np.float32)
    pw2 = np.asarray(inputs["pw2"], dtype=np.float32)
    aW1 = np.asarray(inputs["aW1"], dtype=np.float32)
    ab1 = np.asarray(inputs["ab1"], dtype=np.float32)
    aw2 = np.asarray(inputs["aw2"], dtype=np.float32)

    bf = ml_dtypes.bfloat16
    order_g = np.argsort(-lengths, kind="stable")
    orders = order_g.reshape(PS, NCORES).T
    sorted_len = lengths[orders]
    len_max = sorted_len.max(axis=0)
    blocks = _make_blocks(len_max, 2)

    w1_np = np.ascontiguousarray(
        pW1.reshape(KC, 128, H).transpose(1, 0, 2).reshape(128, KC * H)
    ).astype(bf)
    w2_np = np.ascontiguousarray(pw2.reshape(HC, 128).T).astype(bf)
    b1_np = np.ascontiguousarray(pb1.reshape(HC, 128).T).astype(np.float32)
    aw1_np = np.ascontiguousarray(
        aW1.reshape(KC, 128, H).transpose(1, 0, 2).reshape(128, KC * H)
    ).astype(np.float32)
    ab1_np = np.ascontiguousarray(ab1.reshape(HC, 128).T).astype(np.float32)
    aw2_np = np.ascontiguousarray(aw2.reshape(HC, 128).T).astype(np.float32)
    one1 = np.ones((1, 1), dtype=bf)

    c0 = float(np.maximum(pb1, 0.0) @ pw2)
    ec0 = float(np.exp(c0))

    ar = np.arange(LMAX + 4)
    in_maps = []
    for c in range(NCORES):
        xc = x[orders[c]]
        lc = sorted_len[c]
        xr_parts = []
        npad_vals = np.empty(PS, dtype=np.float32)
        p = 0
        for (bp, cap) in blocks:
            lb = lc[p : p + bp]
            ccap = min(cap, LMAX)
            xblk = xc[p : p + bp, :ccap, :]
            mask = ar[None, :ccap, None] < lb[:, None, None]
            xblk = np.where(mask, xblk, 0.0).astype(bf)
            if ccap < cap:
                pad = np.zeros((bp, cap - ccap, D), dtype=bf)
                xblk = np.concatenate([xblk, pad], axis=1)
            xb_t = (
                xblk.reshape(bp, cap, KC, 128)
                .transpose(2, 3, 0, 1)
                .reshape(KC, 128, bp * cap)
            )
            xr_parts.append(xb_t.ravel())
            npad_vals[p : p + bp] = (cap - lb).astype(np.float32) * ec0
            p += bp
        npad_np = np.broadcast_to(npad_vals, (128, PS)).copy()
        in_maps.append(
            {
                "xb": np.concatenate(xr_parts),
                "npad": npad_np,
                "w1": w1_np,
                "w2": w2_np,
                "b1": b1_np,
                "aw1": aw1_np,
                "ab1": ab1_np,
                "aw2": aw2_np,
                "one1_bf": one1,
            }
        )
    return blocks, in_maps


def _kernel_legacy(inputs):
    global LAST_RESULT
    blocks, in_maps = _prep_legacy(inputs)
    key = ("legacy", blocks)
    if key not in _PROG_CACHE:
        _PROG_CACHE[key] = _build_program_legacy(blocks)
    nc = _PROG_CACHE[key]

    res = bass_utils.run_bass_kernel_spmd(
        nc, in_maps, core_ids=list(range(NCORES)), **_TRACE_KW
    )
    LAST_RESULT = res

    stats = np.stack([r["out_stats"] for r in res.results])   # [8, 1, 2]
    total = float(stats[:, 0, 0].sum())
    vec = np.zeros((128, KC), dtype=np.float64)
    for r in res.results:
        pf = r["out_pf"].reshape(128, KC, PS)
        ea = r["out_ea"].reshape(PS)
        vec += (pf.astype(np.float64) * ea[None, None, :]).sum(axis=2)
    user = np.ascontiguousarray(vec.T).reshape(D) / total
    return user.astype(np.float32)


def kernel(**inputs):
    pb1 = np.asarray(inputs["pb1"], dtype=np.float32)
    ab1 = np.asarray(inputs["ab1"], dtype=np.float32)
    if np.any(pb1 != 0.0) or np.any(ab1 != 0.0):
        return _kernel_legacy(inputs)
    return _kernel_v2(inputs)
